# revision 41
# baseline (speedup 1.0000x reference)
"""Trainium2 Bass kernel for nn_BCErrorCNN (dense_cnn).

Network (per sample, input [17, 9]):
  Conv1D(128, k=3, relu) -> [15, 128]   (position 14 dead: never consumed)
  LocallyConnected1D(128, k=3, relu) -> [13, 128]  (position 12 dead)
  MaxPool1D(2) -> [6, 128]
  LocallyConnected1D(128, k=3, relu) -> [4, 128]
  GlobalAvgPool -> [128]; Dense(100, relu); Dense(1, sigmoid)

Sharding: pure data parallelism, batch 32768 -> 8 cores x 4096.

On-chip layout: activations are [feature(partition), batch(free)], fp32
with float32r matmuls. Each 128-sample group is transposed on the PE
(feature rows 0..127 as TA, rows 25..152 as TB). The conv runs as 14
dense K=128 matmuls against host-side zero-padded weights (wpad), so no
strip-building DMAs are needed: position l reads TA (l<=11) or TB
(l=12,13) directly. lc2's outputs are never materialized: the relu
evacuations accumulate the global-average sum via a scalar_tensor_tensor
chain, and d1_w is pre-scaled by 1/4. Per-tile sigmoid + output DMA
keeps the tail off the critical path. Batch tiles are software-pipelined
with a 1-tile skew (conv of tile i+1 before lc1 of tile i).
"""

import functools

import numpy as np

# ---- constants (hardcoded per problem spec) --------------------------------
N_CORES = 8
B_FULL = 32768
BC = B_FULL // N_CORES  # per-core batch
NB = 512                # batch tile (columns per matmul)
NT = BC // NB           # batch tiles per core
LIN, CIN, F = 17, 9, 128
FEAT = LIN * CIN        # 153
NPOS = 14               # conv positions actually needed (0..13)
NL1 = 12                # lc1 positions needed (0..11)
NPOOL = 6
NL2 = 4
ND1 = 100


def _build_program(nt=NT, flags=(True, True, True, True)):
    import concourse.tile as tile
    from concourse import bacc, mybir
    from concourse.masks import make_identity
    from concourse.tile import add_dep_helper

    cbz, lc1z, lc2z, dbz = flags
    F32 = mybir.dt.float32
    F32R = mybir.dt.float32r
    AF = mybir.ActivationFunctionType
    ALU = mybir.AluOpType

    bc = nt * NB
    nc = bacc.Bacc("TRN2", target_bir_lowering=False, debug=False,
                   num_devices=N_CORES)

    x = nc.dram_tensor("x", [bc * FEAT], F32, kind="ExternalInput").ap()
    wp = nc.dram_tensor("wp", [F, NPOS * F], F32, kind="ExternalInput").ap()
    w1 = nc.dram_tensor("w1", [F, NL1 * 3 * F], F32, kind="ExternalInput").ap()
    w2 = nc.dram_tensor("w2", [F, NL2 * 3 * F], F32, kind="ExternalInput").ap()
    wd1 = nc.dram_tensor("wd1", [F, ND1], F32, kind="ExternalInput").ap()
    wd2 = nc.dram_tensor("wd2", [ND1, 1], F32, kind="ExternalInput").ap()
    cb = nc.dram_tensor("cb", [F, 1], F32, kind="ExternalInput").ap()
    b1 = nc.dram_tensor("b1", [F, NL1], F32, kind="ExternalInput").ap()
    b2 = nc.dram_tensor("b2", [F, NL2], F32, kind="ExternalInput").ap()
    db = nc.dram_tensor("db", [ND1, 1], F32, kind="ExternalInput").ap()
    y = nc.dram_tensor("y", [bc], F32, kind="ExternalOutput").ap()

    def r(ap):
        return ap.bitcast(F32R)

    with tile.TileContext(nc) as tc:
        with (
            tc.tile_pool(name="const", bufs=1) as cpool,
            tc.tile_pool(name="xg", bufs=3) as xpool,
            tc.tile_pool(name="sg", bufs=2) as spool,
            tc.tile_pool(name="h", bufs=3) as hpool,
            tc.tile_pool(name="m", bufs=2) as mpool,
            tc.tile_pool(name="sa", bufs=8) as sapool,
            tc.tile_pool(name="s3", bufs=2) as s3pool,
            tc.tile_pool(name="yo", bufs=2) as ypool,
            tc.tile_pool(name="psT", bufs=1, space="PSUM") as psT,
            tc.tile_pool(name="psC", bufs=2, space="PSUM") as psC,
            tc.tile_pool(name="psL", bufs=3, space="PSUM") as psL,
        ):
            # ---- identity + first input tiles before heavy weight DMAs --
            ident = cpool.tile([128, 128], F32)
            make_identity(nc, ident[:])
            identr = cpool.tile([128, 128], F32)
            nc.vector.tensor_copy(identr[:].bitcast(F32R), ident[:])

            # HAM warm-up: keep the PE streaming through the X0 DMA wait so
            # the 4096-cycle activity window unthrottles the clock (1.2 ->
            # 2.4 GHz) before real matmuls start. Result is never read.
            pwarm = psT.tile([128, NB], F32, tag="T", name="pwarm")
            for i in range(24):
                nc.tensor.matmul(pwarm[:, 0:128], r(identr[:]),
                                 r(identr[:]),
                                 start=(i == 0), stop=(i == 23))

            X_dma = {}

            def load_X(it):
                # X[p, g*153+f] = x[(it*512+g*128+p)*153+f], loaded as two
                # half-tiles so the first transposes start one DMA earlier
                Xa = xpool.tile([128, 2 * FEAT], F32, tag="Xa",
                                name=f"Xa{it}")
                Xb = xpool.tile([128, 2 * FEAT], F32, tag="Xb",
                                name=f"Xb{it}")
                for h, Xh in ((0, Xa), (1, Xb)):
                    base = (it * 512 + h * 256) * FEAT
                    src = x[base:base + 1].copy()
                    src.ap = src.ap[:0] + [[FEAT, 128], [FEAT * 128, 2],
                                           [1, FEAT]]
                    dst = Xh[:, 0:1].copy()
                    dst.ap = dst.ap[:1] + [[FEAT, 2], [1, FEAT]]
                    d = nc.sync.dma_start(dst.bitcast(F32R),
                                          src.bitcast(F32R))
                    if h == 0:
                        X_dma[it] = d
                return (Xa, Xb)

            X_pre = {0: load_X(0)}
            if nt > 1:
                X_pre[1] = load_X(1)

            # conv weights, zero-padded to K=128 (host-prepared); issued
            # right away -- needed by the first conv matmuls
            wpt = cpool.tile([128, NPOS * F], F32)
            nc.scalar.dma_start(wpt[:].bitcast(F32R), wp[:].bitcast(F32R))

            # lc/dense weights host-packed to the exact SBUF layout
            # ([128, blocks*128] contiguous -> cheap 128-row DMA issues).
            # Their dma_starts are deferred into produce(0)/produce(1) so
            # the X0/X1 input transfers win the DMA engines at startup.
            w1t = cpool.tile([128, NL1 * 3 * F], F32)
            w2t = cpool.tile([128, NL2 * 3 * F], F32)
            wd1t = cpool.tile([128, ND1], F32)
            wd2t = cpool.tile([ND1, 1], F32)
            cbt = b1t = b2t = dbt = None
            if not cbz:
                cbt = cpool.tile([F, 1], F32)
                nc.scalar.dma_start(cbt[:], cb[:])

            def load_weights_a():
                # big w1 transfer waits for X0 so the input tile wins the
                # DMA engines and the first transposes start early
                half = NL1 * 3 * F // 2
                d = nc.scalar.dma_start(w1t[:, 0:half].bitcast(F32R),
                                        w1[:, 0:half].bitcast(F32R))
                add_dep_helper(d.ins, X_dma[0].ins, sync=True,
                               reason="w1 transfer after X0 lands")
                d = nc.scalar.dma_start(w1t[:, half:].bitcast(F32R),
                                        w1[:, half:].bitcast(F32R))
                add_dep_helper(d.ins, X_dma[0].ins, sync=True,
                               reason="w1 transfer after X0 lands")
                if not lc1z:
                    nonlocal b1t
                    b1t = cpool.tile([F, NL1], F32)
                    nc.scalar.dma_start(b1t[:], b1[:])

            def load_weights_b():
                nonlocal b2t, dbt
                for dst_t, src_t in ((w2t, w2), (wd1t, wd1), (wd2t, wd2)):
                    d = nc.scalar.dma_start(dst_t[:].bitcast(F32R),
                                            src_t[:].bitcast(F32R))
                    add_dep_helper(d.ins, X_dma[1].ins, sync=True,
                                   reason="weight transfer after X1 lands")
                if not lc2z:
                    b2t = cpool.tile([F, NL2], F32)
                    nc.scalar.dma_start(b2t[:], b2[:])
                if not dbz:
                    dbt = cpool.tile([ND1, 1], F32)
                    nc.scalar.dma_start(dbt[:], db[:])

            H_tiles = {}

            def produce(it):
                Xa, Xb = X_pre.pop(it) if it in X_pre else load_X(it)
                if it + 2 < nt and it + 2 not in X_pre:
                    X_pre[it + 2] = load_X(it + 2)
                if it == 0:
                    load_weights_a()
                elif it == 1:
                    load_weights_b()

                # ---- transposes: TA = rows 0..127, TB = rows 25..152 ---
                TA = spool.tile([128, NB], F32, tag="TA", name=f"TA{it}")
                TB = spool.tile([128, NB], F32, tag="TB", name=f"TB{it}")
                for dst_s, off in ((TA, 0), (TB, 25)):
                    pT = psT.tile([128, NB], F32, tag="T")
                    for g in range(4):
                        Xh = Xa if g < 2 else Xb
                        gc = g % 2
                        nc.tensor.transpose(
                            r(pT[:, g * 128:(g + 1) * 128]),
                            r(Xh[:, gc * FEAT + off:gc * FEAT + off + 128]),
                            r(identr[:]))
                    nc.vector.tensor_copy(dst_s[:].bitcast(F32R), pT[:])

                # ---- conv: 14 positions, dense K=128 vs zero-padded w --
                H = hpool.tile([128, NPOS * NB], F32, tag="H", name=f"H{it}")
                for a in range(NPOS // 2):
                    pC = psC.tile([128, 1024], F32, tag="C",
                                  name=f"pC{it}_{a}")
                    for d in range(2):
                        p = 2 * a + d
                        mov = TA if p <= 11 else TB
                        nc.tensor.matmul(
                            pC[:, d * NB:(d + 1) * NB],
                            r(wpt[:, p * F:(p + 1) * F]),
                            r(mov[:]),
                            start=True, stop=True)
                    # evac split into halves across ACT and DVE so each
                    # psC pair drains in ~0.6us instead of ~1.1us serial.
                    # During pipeline fill (tiles 0-1) DVE has no lc/pool
                    # work yet, so split 7/7 there instead of 9/5.
                    for d in range(2):
                        hdst = H[:, (2 * a + d) * NB:
                                 (2 * a + d + 1) * NB].bitcast(F32R)
                        psrc = pC[:, d * NB:(d + 1) * NB]
                        if it <= 1:
                            on_dve = (d == 1)
                        else:
                            on_dve = (a, d) in ((0, 1), (1, 1), (3, 1),
                                                (4, 1), (6, 1))
                        if on_dve and cbz:
                            nc.vector.tensor_scalar_max(hdst, psrc, 0.0)
                        elif on_dve:
                            nc.vector.tensor_scalar(
                                hdst, psrc, cbt[:], 0.0,
                                op0=ALU.add, op1=ALU.max)
                        else:
                            nc.scalar.activation(
                                hdst, psrc, AF.Relu,
                                bias=0.0 if cbz else cbt[:])
                H_tiles[it] = H

            Sacc_tiles = {}

            def consume_lc(it):
                H = H_tiles.pop(it)
                # ---- lc1 (12 positions) + fused maxpool+relu ----------
                # max is associative: max(relu(a), b) == relu(max(a, b)),
                # so evac even psum to E, then max(E, odd psum) on DVE.
                M = mpool.tile([128, NPOOL * NB], F32, tag="M")
                for t in range(NPOOL):
                    pair = []
                    E = spool.tile([128, NB], F32, tag="E", name=f"E{it}_{t}")
                    for d in range(2):
                        l = 2 * t + d
                        ps = psL.tile([128, NB], F32, tag="L")
                        for k in range(3):
                            nc.tensor.matmul(
                                ps[:],
                                r(w1t[:, (l * 3 + k) * F:(l * 3 + k + 1) * F]),
                                r(H[:, (l + k) * NB:(l + k + 1) * NB]),
                                start=(k == 0), stop=(k == 2))
                        pair.append(ps)
                        if d == 0:
                            # evac even psum immediately (t=0 on DVE for
                            # engine balance, rest on ACT)
                            if t == 0 and lc1z:
                                nc.vector.tensor_scalar_max(E[:], ps[:], 0.0)
                            elif t == 0:
                                nc.vector.tensor_scalar(
                                    E[:], ps[:], b1t[:, 2 * t:2 * t + 1],
                                    0.0, op0=ALU.add, op1=ALU.max)
                            else:
                                bias = (0.0 if lc1z
                                        else b1t[:, 2 * t:2 * t + 1])
                                nc.scalar.activation(E[:], ps[:], AF.Relu,
                                                     bias=bias)
                    mdst = M[:, t * NB:(t + 1) * NB].bitcast(F32R)
                    if lc1z:
                        nc.vector.tensor_tensor(mdst, E[:], pair[1][:],
                                                op=ALU.max)
                    else:
                        nc.vector.scalar_tensor_tensor(
                            mdst, pair[1][:], b1t[:, 2 * t + 1:2 * t + 2],
                            E[:], op0=ALU.add, op1=ALU.max)

                # ---- lc2 (4 positions), fused with global-avg accum ---
                Sacc = None
                for l in range(NL2):
                    ps = psL.tile([128, NB], F32, tag="L")
                    for k in range(3):
                        nc.tensor.matmul(
                            ps[:],
                            r(w2t[:, (l * 3 + k) * F:(l * 3 + k + 1) * F]),
                            r(M[:, (l + k) * NB:(l + k + 1) * NB]),
                            start=(k == 0), stop=(k == 2))
                    Snew = sapool.tile([128, NB], F32, tag="SA",
                                       name=f"SA{it}_{l}")
                    sdst = Snew[:].bitcast(F32R)
                    if l == 0:
                        nc.scalar.activation(
                            sdst, ps[:], AF.Relu,
                            bias=0.0 if lc2z else b2t[:, 0:1])
                    elif lc2z:
                        # Snew = relu(ps) + Sacc in one DVE op
                        nc.vector.scalar_tensor_tensor(
                            sdst, ps[:], 0.0, Sacc[:],
                            op0=ALU.max, op1=ALU.add)
                    else:
                        E2 = spool.tile([128, NB], F32, tag="E2",
                                        name=f"E2{it}_{l}")
                        nc.scalar.activation(E2[:], ps[:], AF.Relu,
                                             bias=b2t[:, l:l + 1])
                        nc.vector.tensor_tensor(sdst, E2[:], Sacc[:],
                                                op=ALU.add)
                    Sacc = Snew
                Sacc_tiles[it] = Sacc

            def consume_d(it):
                # ---- dense1 (wd1 pre-scaled by 1/4) + dense2 + sigmoid -
                # runs one tile behind consume_lc so the lc2 accum chain
                # (serial DVE ops) never gates the dense matmuls
                Sacc = Sacc_tiles.pop(it)
                pD1 = psL.tile([128, NB], F32, tag="L")
                nc.tensor.matmul(pD1[0:ND1, :], r(wd1t[:]), r(Sacc[:]),
                                 start=True, stop=True)
                S3 = s3pool.tile([ND1, NB], F32, tag="S3")
                nc.scalar.activation(S3[:].bitcast(F32R), pD1[0:ND1, :],
                                     AF.Relu, bias=0.0 if dbz else dbt[:])
                pD2 = psL.tile([128, NB], F32, tag="L")
                nc.tensor.matmul(pD2[0:1, :], r(wd2t[:]), r(S3[:]),
                                 start=True, stop=True)
                ysb = ypool.tile([1, NB], F32, tag="Y", name=f"Y{it}")
                nc.scalar.activation(ysb[:], pD2[0:1, :], AF.Sigmoid)
                nc.sync.dma_start(y[it * NB:(it + 1) * NB], ysb[0:1, :])

            # 3-stage pipeline: produce(it) | lc(it-2) | dense(it-3).
            # During produce stalls the scheduler has a full tile of ready
            # lc1 matmuls; the dense stage runs long after its gating chain.
            for it in range(nt + 3):
                if it < nt:
                    produce(it)
                if 2 <= it < nt + 2:
                    consume_lc(it - 2)
                if it >= 3:
                    consume_d(it - 3)

    nc.compile()
    return nc


@functools.lru_cache(maxsize=4)
def _get_program(nt, flags):
    return _build_program(nt, flags)


def _prep_in_maps(inputs, conv_w, conv_b, lc1_w, lc1_b, lc2_w, lc2_b,
                  d1_w, d1_b, d2_w, nt=NT, n_cores=N_CORES):
    bc = nt * NB
    f32 = np.float32
    cbz = not np.any(conv_b)
    lc1z = not np.any(lc1_b[:NL1])
    lc2z = not np.any(lc2_b)
    dbz = not np.any(d1_b)
    # conv weights zero-padded to dense K=128 stationaries per position:
    # l<=11 reads TA (feature rows 0..127), l=12,13 read TB (rows 25..152)
    wc = np.asarray(conv_w, dtype=f32).reshape(27, F)
    wp_np = np.zeros((128, NPOS * F), dtype=f32)
    for l in range(NPOS):
        r0 = 9 * l if l <= 11 else 9 * l - 25
        wp_np[r0:r0 + 27, l * F:(l + 1) * F] = wc
    # host-packed to SBUF layout [r(partition), (l k f)]
    w1_np = np.ascontiguousarray(
        np.asarray(lc1_w[:NL1], dtype=f32).reshape(NL1, 3, F, F)
        .transpose(2, 0, 1, 3).reshape(F, NL1 * 3 * F))
    w2_np = np.ascontiguousarray(
        np.asarray(lc2_w, dtype=f32).reshape(NL2, 3, F, F)
        .transpose(2, 0, 1, 3).reshape(F, NL2 * 3 * F))
    wd1_np = np.ascontiguousarray(d1_w, dtype=f32) * np.float32(0.25)
    wd2_np = np.ascontiguousarray(d2_w.reshape(ND1, 1), dtype=f32)
    cb_np = np.ascontiguousarray(conv_b.reshape(F, 1), dtype=f32)
    b1_np = np.ascontiguousarray(lc1_b[:NL1].T, dtype=f32)
    b2_np = np.ascontiguousarray(lc2_b.T, dtype=f32)
    db_np = np.ascontiguousarray(d1_b.reshape(ND1, 1), dtype=f32)
    shared = dict(wp=wp_np, w1=w1_np, w2=w2_np, wd1=wd1_np, wd2=wd2_np,
                  cb=cb_np, b1=b1_np, b2=b2_np, db=db_np)
    in_maps = []
    for c in range(n_cores):
        shard = np.ascontiguousarray(
            inputs[c * bc:(c + 1) * bc], dtype=f32).reshape(bc * FEAT)
        in_maps.append(dict(shared, x=shard))
    return in_maps, (cbz, lc1z, lc2z, dbz)


def kernel(inputs, conv_w, conv_b, lc1_w, lc1_b, lc2_w, lc2_b,
           d1_w, d1_b, d2_w):
    from concourse.bass_utils import run_bass_kernel_spmd

    in_maps, flags = _prep_in_maps(
        inputs, conv_w, conv_b, lc1_w, lc1_b, lc2_w, lc2_b, d1_w, d1_b, d2_w)
    nc = _get_program(NT, flags)
    res = run_bass_kernel_spmd(nc, in_maps, list(range(N_CORES)))
    out = np.concatenate([res.results[c]["y"] for c in range(N_CORES)])
    return out.reshape(B_FULL, 1).astype(np.float32)


# revision 43
# speedup vs baseline: 1.0055x; 1.0055x over previous
"""Trainium2 Bass kernel for nn_BCErrorCNN (dense_cnn).

Network (per sample, input [17, 9]):
  Conv1D(128, k=3, relu) -> [15, 128]   (position 14 dead: never consumed)
  LocallyConnected1D(128, k=3, relu) -> [13, 128]  (position 12 dead)
  MaxPool1D(2) -> [6, 128]
  LocallyConnected1D(128, k=3, relu) -> [4, 128]
  GlobalAvgPool -> [128]; Dense(100, relu); Dense(1, sigmoid)

Sharding: pure data parallelism, batch 32768 -> 8 cores x 4096.

On-chip layout: activations are [feature(partition), batch(free)], fp32
with float32r matmuls. Each 128-sample group is transposed on the PE
(feature rows 0..127 as TA, rows 25..152 as TB). The conv runs as 14
dense K=128 matmuls against host-side zero-padded weights (wpad), so no
strip-building DMAs are needed: position l reads TA (l<=11) or TB
(l=12,13) directly. lc2's outputs are never materialized: the relu
evacuations accumulate the global-average sum via a scalar_tensor_tensor
chain, and d1_w is pre-scaled by 1/4. Per-tile sigmoid + output DMA
keeps the tail off the critical path. Batch tiles are software-pipelined
with a 1-tile skew (conv of tile i+1 before lc1 of tile i).
"""

import functools

import numpy as np

# ---- constants (hardcoded per problem spec) --------------------------------
N_CORES = 8
B_FULL = 32768
BC = B_FULL // N_CORES  # per-core batch
NB = 512                # batch tile (columns per matmul)
NT = BC // NB           # batch tiles per core
LIN, CIN, F = 17, 9, 128
FEAT = LIN * CIN        # 153
NPOS = 14               # conv positions actually needed (0..13)
NL1 = 12                # lc1 positions needed (0..11)
NPOOL = 6
NL2 = 4
ND1 = 100


def _build_program(nt=NT, flags=(True, True, True, True)):
    import concourse.tile as tile
    from concourse import bacc, mybir
    from concourse.masks import make_identity
    from concourse.tile import add_dep_helper

    cbz, lc1z, lc2z, dbz = flags
    F32 = mybir.dt.float32
    F32R = mybir.dt.float32r
    AF = mybir.ActivationFunctionType
    ALU = mybir.AluOpType

    bc = nt * NB
    nc = bacc.Bacc("TRN2", target_bir_lowering=False, debug=False,
                   num_devices=N_CORES)

    x = nc.dram_tensor("x", [bc * FEAT], F32, kind="ExternalInput").ap()
    wp = nc.dram_tensor("wp", [F, NPOS * F], F32, kind="ExternalInput").ap()
    w1 = nc.dram_tensor("w1", [F, NL1 * 3 * F], F32, kind="ExternalInput").ap()
    w2 = nc.dram_tensor("w2", [F, NL2 * 3 * F], F32, kind="ExternalInput").ap()
    wd1 = nc.dram_tensor("wd1", [F, ND1], F32, kind="ExternalInput").ap()
    wd2 = nc.dram_tensor("wd2", [ND1, 1], F32, kind="ExternalInput").ap()
    cb = nc.dram_tensor("cb", [F, 1], F32, kind="ExternalInput").ap()
    b1 = nc.dram_tensor("b1", [F, NL1], F32, kind="ExternalInput").ap()
    b2 = nc.dram_tensor("b2", [F, NL2], F32, kind="ExternalInput").ap()
    db = nc.dram_tensor("db", [ND1, 1], F32, kind="ExternalInput").ap()
    y = nc.dram_tensor("y", [bc], F32, kind="ExternalOutput").ap()

    def r(ap):
        return ap.bitcast(F32R)

    with tile.TileContext(nc) as tc:
        with (
            tc.tile_pool(name="const", bufs=1) as cpool,
            tc.tile_pool(name="xg", bufs=3) as xpool,
            tc.tile_pool(name="sg", bufs=2) as spool,
            tc.tile_pool(name="h", bufs=3) as hpool,
            tc.tile_pool(name="m", bufs=2) as mpool,
            tc.tile_pool(name="sa", bufs=8) as sapool,
            tc.tile_pool(name="s3", bufs=2) as s3pool,
            tc.tile_pool(name="yo", bufs=2) as ypool,
            tc.tile_pool(name="psT", bufs=1, space="PSUM") as psT,
            tc.tile_pool(name="psC", bufs=2, space="PSUM") as psC,
            tc.tile_pool(name="psL", bufs=3, space="PSUM") as psL,
        ):
            # ---- identity + first input tiles before heavy weight DMAs --
            ident = cpool.tile([128, 128], F32)
            make_identity(nc, ident[:])
            identr = cpool.tile([128, 128], F32)
            nc.vector.tensor_copy(identr[:].bitcast(F32R), ident[:])

            # HAM warm-up: keep the PE streaming through the X0 DMA wait so
            # the 4096-cycle activity window unthrottles the clock (1.2 ->
            # 2.4 GHz) before real matmuls start. Result is never read.
            # 8 x ~427ns (N=128 fp32r is 2 cyc/row + serial LDW) covers the
            # ~3.4us window without queue-blocking the real transposes.
            pwarm = psT.tile([128, NB], F32, tag="T", name="pwarm")
            for i in range(8):
                nc.tensor.matmul(pwarm[:, 0:128], r(identr[:]),
                                 r(identr[:]),
                                 start=(i == 0), stop=(i == 7))

            X_dma = {}

            def load_X(it):
                # X[p, g*153+f] = x[(it*512+g*128+p)*153+f], loaded as two
                # half-tiles so the first transposes start one DMA earlier.
                # At pipeline fill (it<=1) the halves go out on BOTH the
                # sync and scalar queues so their issues don't serialize.
                Xa = xpool.tile([128, 2 * FEAT], F32, tag="Xa",
                                name=f"Xa{it}")
                Xb = xpool.tile([128, 2 * FEAT], F32, tag="Xb",
                                name=f"Xb{it}")
                for h, Xh in ((0, Xa), (1, Xb)):
                    base = (it * 512 + h * 256) * FEAT
                    src = x[base:base + 1].copy()
                    src.ap = src.ap[:0] + [[FEAT, 128], [FEAT * 128, 2],
                                           [1, FEAT]]
                    dst = Xh[:, 0:1].copy()
                    dst.ap = dst.ap[:1] + [[FEAT, 2], [1, FEAT]]
                    eng = nc.scalar if (h == 1 and it <= 1) else nc.sync
                    d = eng.dma_start(dst.bitcast(F32R), src.bitcast(F32R))
                    if h == 0:
                        X_dma[it] = d
                return (Xa, Xb)

            X_pre = {0: load_X(0)}
            if nt > 1:
                X_pre[1] = load_X(1)

            # conv weights, zero-padded to K=128 (host-prepared); issued
            # right away -- needed by the first conv matmuls
            wpt = cpool.tile([128, NPOS * F], F32)
            nc.scalar.dma_start(wpt[:].bitcast(F32R), wp[:].bitcast(F32R))

            # lc/dense weights host-packed to the exact SBUF layout
            # ([128, blocks*128] contiguous -> cheap 128-row DMA issues).
            # Their dma_starts are deferred into produce(0)/produce(1) so
            # the X0/X1 input transfers win the DMA engines at startup.
            w1t = cpool.tile([128, NL1 * 3 * F], F32)
            w2t = cpool.tile([128, NL2 * 3 * F], F32)
            wd1t = cpool.tile([128, ND1], F32)
            wd2t = cpool.tile([ND1, 1], F32)
            cbt = b1t = b2t = dbt = None
            if not cbz:
                cbt = cpool.tile([F, 1], F32)
                nc.scalar.dma_start(cbt[:], cb[:])

            def load_weights_a():
                # big w1 transfer waits for X0 so the input tile wins the
                # DMA engines and the first transposes start early
                half = NL1 * 3 * F // 2
                d = nc.scalar.dma_start(w1t[:, 0:half].bitcast(F32R),
                                        w1[:, 0:half].bitcast(F32R))
                add_dep_helper(d.ins, X_dma[0].ins, sync=True,
                               reason="w1 transfer after X0 lands")
                d = nc.scalar.dma_start(w1t[:, half:].bitcast(F32R),
                                        w1[:, half:].bitcast(F32R))
                add_dep_helper(d.ins, X_dma[0].ins, sync=True,
                               reason="w1 transfer after X0 lands")
                if not lc1z:
                    nonlocal b1t
                    b1t = cpool.tile([F, NL1], F32)
                    nc.scalar.dma_start(b1t[:], b1[:])

            def load_weights_b():
                nonlocal b2t, dbt
                for dst_t, src_t in ((w2t, w2), (wd1t, wd1), (wd2t, wd2)):
                    d = nc.scalar.dma_start(dst_t[:].bitcast(F32R),
                                            src_t[:].bitcast(F32R))
                    add_dep_helper(d.ins, X_dma[1].ins, sync=True,
                                   reason="weight transfer after X1 lands")
                if not lc2z:
                    b2t = cpool.tile([F, NL2], F32)
                    nc.scalar.dma_start(b2t[:], b2[:])
                if not dbz:
                    dbt = cpool.tile([ND1, 1], F32)
                    nc.scalar.dma_start(dbt[:], db[:])

            H_tiles = {}

            def produce(it):
                Xa, Xb = X_pre.pop(it) if it in X_pre else load_X(it)
                if it + 2 < nt and it + 2 not in X_pre:
                    X_pre[it + 2] = load_X(it + 2)
                if it == 0:
                    load_weights_a()
                elif it == 1:
                    load_weights_b()

                # ---- transposes: TA = rows 0..127, TB = rows 25..152 ---
                TA = spool.tile([128, NB], F32, tag="TA", name=f"TA{it}")
                TB = spool.tile([128, NB], F32, tag="TB", name=f"TB{it}")
                for dst_s, off in ((TA, 0), (TB, 25)):
                    pT = psT.tile([128, NB], F32, tag="T")
                    for g in range(4):
                        Xh = Xa if g < 2 else Xb
                        gc = g % 2
                        nc.tensor.transpose(
                            r(pT[:, g * 128:(g + 1) * 128]),
                            r(Xh[:, gc * FEAT + off:gc * FEAT + off + 128]),
                            r(identr[:]))
                    nc.vector.tensor_copy(dst_s[:].bitcast(F32R), pT[:])

                # ---- conv: 14 positions, dense K=128 vs zero-padded w --
                H = hpool.tile([128, NPOS * NB], F32, tag="H", name=f"H{it}")
                for a in range(NPOS // 2):
                    pC = psC.tile([128, 1024], F32, tag="C",
                                  name=f"pC{it}_{a}")
                    for d in range(2):
                        p = 2 * a + d
                        mov = TA if p <= 11 else TB
                        nc.tensor.matmul(
                            pC[:, d * NB:(d + 1) * NB],
                            r(wpt[:, p * F:(p + 1) * F]),
                            r(mov[:]),
                            start=True, stop=True)
                    # evac split into halves across ACT and DVE so each
                    # psC pair drains in ~0.6us instead of ~1.1us serial.
                    # During pipeline fill (tiles 0-1) DVE has no lc/pool
                    # work yet, so split 7/7 there instead of 9/5.
                    for d in range(2):
                        hdst = H[:, (2 * a + d) * NB:
                                 (2 * a + d + 1) * NB].bitcast(F32R)
                        psrc = pC[:, d * NB:(d + 1) * NB]
                        if it <= 1:
                            on_dve = (d == 1)
                        else:
                            on_dve = (a, d) in ((0, 1), (1, 1), (3, 1),
                                                (4, 1), (6, 1))
                        if on_dve and cbz:
                            nc.vector.tensor_scalar_max(hdst, psrc, 0.0)
                        elif on_dve:
                            nc.vector.tensor_scalar(
                                hdst, psrc, cbt[:], 0.0,
                                op0=ALU.add, op1=ALU.max)
                        else:
                            nc.scalar.activation(
                                hdst, psrc, AF.Relu,
                                bias=0.0 if cbz else cbt[:])
                H_tiles[it] = H

            Sacc_tiles = {}

            def consume_lc(it):
                H = H_tiles.pop(it)
                # ---- lc1 (12 positions) + fused maxpool+relu ----------
                # max is associative: max(relu(a), b) == relu(max(a, b)),
                # so evac even psum to E, then max(E, odd psum) on DVE.
                M = mpool.tile([128, NPOOL * NB], F32, tag="M")
                for t in range(NPOOL):
                    pair = []
                    E = spool.tile([128, NB], F32, tag="E", name=f"E{it}_{t}")
                    for d in range(2):
                        l = 2 * t + d
                        ps = psL.tile([128, NB], F32, tag="L")
                        for k in range(3):
                            nc.tensor.matmul(
                                ps[:],
                                r(w1t[:, (l * 3 + k) * F:(l * 3 + k + 1) * F]),
                                r(H[:, (l + k) * NB:(l + k + 1) * NB]),
                                start=(k == 0), stop=(k == 2))
                        pair.append(ps)
                        if d == 0:
                            # evac even psum immediately (t=0 on DVE for
                            # engine balance, rest on ACT)
                            if t == 0 and lc1z:
                                nc.vector.tensor_scalar_max(E[:], ps[:], 0.0)
                            elif t == 0:
                                nc.vector.tensor_scalar(
                                    E[:], ps[:], b1t[:, 2 * t:2 * t + 1],
                                    0.0, op0=ALU.add, op1=ALU.max)
                            else:
                                bias = (0.0 if lc1z
                                        else b1t[:, 2 * t:2 * t + 1])
                                nc.scalar.activation(E[:], ps[:], AF.Relu,
                                                     bias=bias)
                    mdst = M[:, t * NB:(t + 1) * NB].bitcast(F32R)
                    if lc1z:
                        nc.vector.tensor_tensor(mdst, E[:], pair[1][:],
                                                op=ALU.max)
                    else:
                        nc.vector.scalar_tensor_tensor(
                            mdst, pair[1][:], b1t[:, 2 * t + 1:2 * t + 2],
                            E[:], op0=ALU.add, op1=ALU.max)

                # ---- lc2 (4 positions), fused with global-avg accum ---
                Sacc = None
                for l in range(NL2):
                    ps = psL.tile([128, NB], F32, tag="L")
                    for k in range(3):
                        nc.tensor.matmul(
                            ps[:],
                            r(w2t[:, (l * 3 + k) * F:(l * 3 + k + 1) * F]),
                            r(M[:, (l + k) * NB:(l + k + 1) * NB]),
                            start=(k == 0), stop=(k == 2))
                    Snew = sapool.tile([128, NB], F32, tag="SA",
                                       name=f"SA{it}_{l}")
                    sdst = Snew[:].bitcast(F32R)
                    if l == 0:
                        nc.scalar.activation(
                            sdst, ps[:], AF.Relu,
                            bias=0.0 if lc2z else b2t[:, 0:1])
                    elif lc2z:
                        # Snew = relu(ps) + Sacc in one DVE op
                        nc.vector.scalar_tensor_tensor(
                            sdst, ps[:], 0.0, Sacc[:],
                            op0=ALU.max, op1=ALU.add)
                    else:
                        E2 = spool.tile([128, NB], F32, tag="E2",
                                        name=f"E2{it}_{l}")
                        nc.scalar.activation(E2[:], ps[:], AF.Relu,
                                             bias=b2t[:, l:l + 1])
                        nc.vector.tensor_tensor(sdst, E2[:], Sacc[:],
                                                op=ALU.add)
                    Sacc = Snew
                Sacc_tiles[it] = Sacc

            def consume_d(it):
                # ---- dense1 (wd1 pre-scaled by 1/4) + dense2 + sigmoid -
                # runs one tile behind consume_lc so the lc2 accum chain
                # (serial DVE ops) never gates the dense matmuls
                Sacc = Sacc_tiles.pop(it)
                pD1 = psL.tile([128, NB], F32, tag="L")
                nc.tensor.matmul(pD1[0:ND1, :], r(wd1t[:]), r(Sacc[:]),
                                 start=True, stop=True)
                S3 = s3pool.tile([ND1, NB], F32, tag="S3")
                nc.scalar.activation(S3[:].bitcast(F32R), pD1[0:ND1, :],
                                     AF.Relu, bias=0.0 if dbz else dbt[:])
                pD2 = psL.tile([128, NB], F32, tag="L")
                nc.tensor.matmul(pD2[0:1, :], r(wd2t[:]), r(S3[:]),
                                 start=True, stop=True)
                ysb = ypool.tile([1, NB], F32, tag="Y", name=f"Y{it}")
                nc.scalar.activation(ysb[:], pD2[0:1, :], AF.Sigmoid)
                nc.sync.dma_start(y[it * NB:(it + 1) * NB], ysb[0:1, :])

            # 3-stage pipeline: produce(it) | lc(it-2) | dense(it-3).
            # During produce stalls the scheduler has a full tile of ready
            # lc1 matmuls; the dense stage runs long after its gating chain.
            for it in range(nt + 3):
                if it < nt:
                    produce(it)
                if 2 <= it < nt + 2:
                    consume_lc(it - 2)
                if it >= 3:
                    consume_d(it - 3)

    nc.compile()
    return nc


@functools.lru_cache(maxsize=4)
def _get_program(nt, flags):
    return _build_program(nt, flags)


def _prep_in_maps(inputs, conv_w, conv_b, lc1_w, lc1_b, lc2_w, lc2_b,
                  d1_w, d1_b, d2_w, nt=NT, n_cores=N_CORES):
    bc = nt * NB
    f32 = np.float32
    cbz = not np.any(conv_b)
    lc1z = not np.any(lc1_b[:NL1])
    lc2z = not np.any(lc2_b)
    dbz = not np.any(d1_b)
    # conv weights zero-padded to dense K=128 stationaries per position:
    # l<=11 reads TA (feature rows 0..127), l=12,13 read TB (rows 25..152)
    wc = np.asarray(conv_w, dtype=f32).reshape(27, F)
    wp_np = np.zeros((128, NPOS * F), dtype=f32)
    for l in range(NPOS):
        r0 = 9 * l if l <= 11 else 9 * l - 25
        wp_np[r0:r0 + 27, l * F:(l + 1) * F] = wc
    # host-packed to SBUF layout [r(partition), (l k f)]
    w1_np = np.ascontiguousarray(
        np.asarray(lc1_w[:NL1], dtype=f32).reshape(NL1, 3, F, F)
        .transpose(2, 0, 1, 3).reshape(F, NL1 * 3 * F))
    w2_np = np.ascontiguousarray(
        np.asarray(lc2_w, dtype=f32).reshape(NL2, 3, F, F)
        .transpose(2, 0, 1, 3).reshape(F, NL2 * 3 * F))
    wd1_np = np.ascontiguousarray(d1_w, dtype=f32) * np.float32(0.25)
    wd2_np = np.ascontiguousarray(d2_w.reshape(ND1, 1), dtype=f32)
    cb_np = np.ascontiguousarray(conv_b.reshape(F, 1), dtype=f32)
    b1_np = np.ascontiguousarray(lc1_b[:NL1].T, dtype=f32)
    b2_np = np.ascontiguousarray(lc2_b.T, dtype=f32)
    db_np = np.ascontiguousarray(d1_b.reshape(ND1, 1), dtype=f32)
    shared = dict(wp=wp_np, w1=w1_np, w2=w2_np, wd1=wd1_np, wd2=wd2_np,
                  cb=cb_np, b1=b1_np, b2=b2_np, db=db_np)
    in_maps = []
    for c in range(n_cores):
        shard = np.ascontiguousarray(
            inputs[c * bc:(c + 1) * bc], dtype=f32).reshape(bc * FEAT)
        in_maps.append(dict(shared, x=shard))
    return in_maps, (cbz, lc1z, lc2z, dbz)


def kernel(inputs, conv_w, conv_b, lc1_w, lc1_b, lc2_w, lc2_b,
           d1_w, d1_b, d2_w):
    from concourse.bass_utils import run_bass_kernel_spmd

    in_maps, flags = _prep_in_maps(
        inputs, conv_w, conv_b, lc1_w, lc1_b, lc2_w, lc2_b, d1_w, d1_b, d2_w)
    nc = _get_program(NT, flags)
    res = run_bass_kernel_spmd(nc, in_maps, list(range(N_CORES)))
    out = np.concatenate([res.results[c]["y"] for c in range(N_CORES)])
    return out.reshape(B_FULL, 1).astype(np.float32)
